# revision 17
# baseline (speedup 1.0000x reference)
"""Trainium2 Bass kernel for nn_EntropyCalculator (per-row histogram entropy).

x: [262144, 64] int32, values in [0, 40). Output: [262144, 1] float32 per-row
entropy of the value histogram: -sum_v p_v*log(p_v + 1e-8), p = c/(64+1e-8).

Strategy (per core, pure data parallel over 8 cores):
  The histogram over 40 bins is computed with 14 "limb" passes. Limb g
  packs the counts of values {3g, 3g+1, 3g+2} into one fp32 accumulator
  as c0 + 128*c1 + c2/128 (exact: counts <= 64 -> 7 bits per digit,
  21 bits + sign headroom < fp32's 24-bit mantissa). The per-element
  contribution (1, 128, or 1/128 inside the window, 0 outside) is one
  fused custom DVE op: relu(1 + a*t + b*t^2), t = x - 3g, a downward
  parabola that hits exactly (1, 128, 1/128) at t = 0,1,2 and is
  negative at every other integer in [-39, 41]. The same op folds in a
  prefix scan along the free dim; per-row sums are recovered by strided
  differences of the prefix at row boundaries (rows of 64 elements; scan
  chunks capped at 16 rows to keep every partial sum exact in fp32).
  Digits are decoded with exact rint(x*s - 0.25) ops (magic-number
  rounding with +2^23), and the entropy tail is ACT-Log + a fused
  multiply-scan.
"""

import numpy as np

VOCAB = 40
L = 64
B = 262144
NCORES = 8
ROWS_PC = B // NCORES          # 32768 rows per core
P = 128                        # SBUF partitions
RPP = ROWS_PC // P             # 256 rows per partition
RC = 32                        # rows per partition per chunk
NCHUNK = RPP // RC             # 8 chunks
SCANROWS = 16                  # rows per scan instruction (fp32 exactness cap)
NSUB = RC // SCANROWS          # 2 scan sub-chunks per chunk
NLIMB = 14
EPS = 1e-8
S_PRIME = 64.0 + EPS

# parabola through (1, 128, 1/128) at t=0,1,2; negative at all other ints
A_C = 254.49609375
B_C = -127.49609375
MAGIC = 8388608.0              # 2^23: rint via (x + 2^23) - 2^23

_RUNNER = None


def _register_ops():
    import concourse.dve_ops as dve_ops
    from concourse.dve_spec import (
        Spec, Src0, Src1, C0, C1, C2, One, scan, AluOp, lower, _has_src1, sq,
        relu,
    )
    from concourse.dve_uop import DveOpSpec

    def reg(name, spec, subdim=False):
        for op in dve_ops.OPS:
            if op.name == name:
                return op
        row = dve_ops._CUSTOM_DVE_ROW_BASE + len(dve_ops.OPS)
        assert row < 0x20, "out of custom-DVE opcode rows"
        shas = {}
        for ver in ("v3", "v4"):
            s = DveOpSpec(name=name, opcode=row, uops=lower(spec, ver=ver),
                          rd1_en=_has_src1(spec))
            shas[ver] = s.sha(ver)
        op = dve_ops.DveOp(name, spec, subdim=subdim, uops_sha=shas)
        dve_ops.OPS.append(op)
        dve_ops.CUSTOM_DVE_SPECS[name] = spec
        dve_ops._SUB_OPCODE_FOR_NAME[name] = row
        return op

    _t = Src0 - C0

    def _ref_limb(in0, in1, s0, s1, imm2):
        t = in0.astype(np.float64) - s0
        z = np.maximum(1.0 + t * s1 + t * t * imm2, 0.0)
        return np.cumsum(z.reshape(z.shape[0], -1), axis=1).astype(np.float32)

    limb = reg("ENT_LIMB_SCAN", Spec(
        body=scan(AluOp.ADD, relu(One + _t * C1 + sq(_t) * C2)),
        reference=_ref_limb))

    def _ref_rint(in0, in1, s0, s1, imm2):
        y = (in0.astype(np.float32) * np.float32(s0)) - np.float32(s1)
        return ((y + np.float32(imm2)) - np.float32(imm2)).astype(np.float32)

    rint = reg("ENT_RINT_AFFINE", Spec(
        body=(Src0 * C0 - C1 + C2) - C2,
        reference=_ref_rint))

    def _ref_dot(in0, in1, s0, s1, imm2):
        z = in0.astype(np.float64) * in1.astype(np.float64)
        return np.cumsum(z.reshape(z.shape[0], -1), axis=1).astype(np.float32)

    dot = reg("ENT_DOT_SCAN", Spec(
        body=scan(AluOp.ADD, Src0 * Src1),
        reference=_ref_dot))

    return limb, rint, dot


def _build_nc(repeat=1):
    from contextlib import ExitStack
    import concourse.bacc as bacc
    import concourse.mybir as mybir
    from concourse.tile import TileContext

    LIMB, RINT, DOT = _register_ops()
    dt = mybir.dt
    Alu = mybir.AluOpType

    nc = bacc.Bacc()
    x = nc.dram_tensor("x", [ROWS_PC, L], dt.int32, kind="ExternalInput")
    y = nc.dram_tensor("y", [ROWS_PC, 1], dt.float32, kind="ExternalOutput")

    # partition p owns rows [p*RPP, (p+1)*RPP); chunk c covers rows c*RC..+RC
    xv = x[:].rearrange("(p c r) l -> p c (r l)", p=P, c=NCHUNK)   # [P, NCHUNK, RC*L]
    yv = y[:].rearrange("(p c r) o -> p c (r o)", p=P, c=NCHUNK)   # [P, NCHUNK, RC]

    NA = RC * NLIMB            # 448 accumulators per partition per chunk
    inv_sp = float(1.0 / S_PRIME)

    with TileContext(nc) as tc:
        with ExitStack() as ctx:
            xpool = ctx.enter_context(tc.tile_pool(name="xp", bufs=3))
            ppool = ctx.enter_context(tc.tile_pool(name="pp", bufs=3))
            apool = ctx.enter_context(tc.tile_pool(name="ap", bufs=2))
            dpool = ctx.enter_context(tc.tile_pool(name="dp", bufs=2))
            epool = ctx.enter_context(tc.tile_pool(name="ep", bufs=2))
            singles = ctx.enter_context(tc.tile_pool(name="sg", bufs=1))

            t_eps = singles.tile([P, 1], dt.float32)
            nc.vector.memset(t_eps[:], EPS)
            t_inv = singles.tile([P, 1], dt.float32)
            nc.vector.memset(t_inv[:], inv_sp)
            t_inv128 = singles.tile([P, 1], dt.float32)
            nc.vector.memset(t_inv128[:], float(128.0 / S_PRIME))

            from contextlib import nullcontext
            repctx = tc.For_i(0, repeat, 1) if repeat > 1 else nullcontext()
            with repctx:
              for c in range(NCHUNK):
                xt = xpool.tile([P, RC * L], dt.int32, tag="x")
                nc.sync.dma_start(out=xt[:], in_=xv[:, c, :])

                Ab = apool.tile([P, NSUB, SCANROWS, NLIMB], dt.float32, tag="A")

                for g in range(NLIMB):
                    pref = ppool.tile([P, NSUB, SCANROWS, L], dt.float32,
                                      tag="pref")
                    prefF = pref[:].rearrange("p s r l -> p (s r l)")
                    for s in range(NSUB):
                        nc.vector._custom_dve(
                            LIMB,
                            out=prefF[:, s * SCANROWS * L:(s + 1) * SCANROWS * L],
                            in0=xt[:, s * SCANROWS * L:(s + 1) * SCANROWS * L],
                            s0=float(3 * g), s1=A_C, imm2=B_C)
                    # row sums: first row of each scan sub-chunk is the raw
                    # prefix at l=63; later rows are boundary differences.
                    nc.scalar.copy(Ab[:, :, 0, g], pref[:, :, 0, 63])
                    nc.gpsimd.tensor_tensor(
                        out=Ab[:, :, 1:, g],
                        in0=pref[:, :, 1:, 63],
                        in1=pref[:, :, :-1, 63],
                        op=Alu.subtract)

                Af = Ab[:].rearrange("p s r g -> p (s r g)")       # [P, NA]
                ri = dpool.tile([P, NA], dt.float32, tag="ri")     # c0 + 128*c1
                dd = dpool.tile([P, NA], dt.float32, tag="dd")     # c2/128
                c1 = dpool.tile([P, NA], dt.float32, tag="c1")
                c0 = dpool.tile([P, NA], dt.float32, tag="c0")
                nc.vector._custom_dve(RINT, out=ri[:], in0=Af,
                                      s0=1.0, s1=0.25, imm2=MAGIC)
                nc.gpsimd.tensor_tensor(out=dd[:], in0=Af, in1=ri[:],
                                        op=Alu.subtract)
                nc.vector._custom_dve(RINT, out=c1[:], in0=ri[:],
                                      s0=0.0078125, s1=0.25, imm2=MAGIC)
                nc.vector.scalar_tensor_tensor(
                    out=c0[:], in0=c1[:], scalar=-128.0, in1=ri[:],
                    op0=Alu.mult, op1=Alu.add)

                # u_i = log(c_i/S' + eps); for the dd lane fold the 128 into
                # the ACT scale and into the final combine instead.
                u0 = dpool.tile([P, NA], dt.float32, tag="u0")
                u1 = dpool.tile([P, NA], dt.float32, tag="u1")
                u2 = dpool.tile([P, NA], dt.float32, tag="u2")
                nc.scalar.activation(u0[:], c0[:],
                                     mybir.ActivationFunctionType.Ln,
                                     bias=t_eps[:], scale=t_inv[:])
                nc.scalar.activation(u1[:], c1[:],
                                     mybir.ActivationFunctionType.Ln,
                                     bias=t_eps[:], scale=t_inv[:])
                nc.scalar.activation(u2[:], dd[:],
                                     mybir.ActivationFunctionType.Ln,
                                     bias=t_eps[:], scale=t_inv128[:])

                d0 = dpool.tile([P, RC, NLIMB], dt.float32, tag="d0")
                d1 = dpool.tile([P, RC, NLIMB], dt.float32, tag="d1")
                d2 = dpool.tile([P, RC, NLIMB], dt.float32, tag="d2")
                nc.vector._custom_dve(
                    DOT, out=d0[:].rearrange("p r g -> p (r g)"),
                    in0=c0[:], in1=u0[:])
                nc.vector._custom_dve(
                    DOT, out=d1[:].rearrange("p r g -> p (r g)"),
                    in0=c1[:], in1=u1[:])
                nc.vector._custom_dve(
                    DOT, out=d2[:].rearrange("p r g -> p (r g)"),
                    in0=dd[:], in1=u2[:])

                # per-row sums from scan ends: S[r] = d[r,13] - d[r-1,13]
                e0 = epool.tile([P, RC], dt.float32, tag="e0")
                e1 = epool.tile([P, RC], dt.float32, tag="e1")
                e2 = epool.tile([P, RC], dt.float32, tag="e2")
                for (dx, ex) in ((d0, e0), (d1, e1), (d2, e2)):
                    nc.scalar.copy(ex[:, 0:1], dx[:, 0:1, NLIMB - 1])
                    nc.gpsimd.tensor_tensor(
                        out=ex[:, 1:], in0=dx[:, 1:, NLIMB - 1],
                        in1=dx[:, :-1, NLIMB - 1], op=Alu.subtract)

                # entropy = -(E0 + E1 + 128*E2)/S'
                acc = epool.tile([P, RC], dt.float32, tag="acc")
                nc.vector.scalar_tensor_tensor(
                    out=acc[:], in0=e2[:], scalar=128.0, in1=e1[:],
                    op0=Alu.mult, op1=Alu.add)
                nc.gpsimd.tensor_tensor(out=acc[:], in0=acc[:], in1=e0[:],
                                        op=Alu.add)
                eout = epool.tile([P, RC], dt.float32, tag="eout")
                nc.vector.tensor_scalar_mul(eout[:], acc[:], float(-1.0 / S_PRIME))
                nc.sync.dma_start(out=yv[:, c, :], in_=eout[:])

    nc.finalize()
    return nc


def _build_runner(repeat=1):
    """Cached jitted 8-core runner (modeled on bass2jax.run_bass_via_pjrt,
    but reusing one jitted executable across calls)."""
    import jax
    import jax.numpy as jnp
    from jax.sharding import Mesh, PartitionSpec
    from jax.experimental.shard_map import shard_map
    import concourse.bass2jax as b2j

    nc = _build_nc(repeat=repeat)
    b2j.install_neuronx_cc_hook()

    import concourse.mybir as mybir
    partition_name = (nc.partition_id_tensor.name
                      if nc.partition_id_tensor else None)
    in_names, out_names, out_avals, zero_outs = [], [], [], []
    for alloc in nc.m.functions[0].allocations:
        if not isinstance(alloc, mybir.MemoryLocationSet):
            continue
        name = alloc.memorylocations[0].name
        if alloc.kind == "ExternalInput":
            if name != partition_name:
                in_names.append(name)
        elif alloc.kind == "ExternalOutput":
            shape = tuple(alloc.tensor_shape)
            dtype = mybir.dt.np(alloc.dtype)
            out_names.append(name)
            out_avals.append(jax.core.ShapedArray(shape, dtype))
            zero_outs.append(np.zeros(shape, dtype))
    n_params = len(in_names)
    n_outs = len(out_avals)
    all_in_names = in_names + out_names
    if partition_name is not None:
        all_in_names = all_in_names + [partition_name]

    def _body(*args):
        operands = list(args)
        if partition_name is not None:
            operands.append(b2j.partition_id_tensor())
        outs = b2j._bass_exec_p.bind(
            *operands,
            out_avals=tuple(out_avals),
            in_names=tuple(all_in_names),
            out_names=tuple(out_names),
            lowering_input_output_aliases=(),
            sim_require_finite=True,
            sim_require_nnan=True,
            nc=nc,
        )
        return tuple(outs)

    devices = jax.devices()[:NCORES]
    mesh = Mesh(np.asarray(devices), ("core",))
    sharded = jax.jit(
        shard_map(_body, mesh=mesh,
                  in_specs=(PartitionSpec("core"),) * (n_params + n_outs),
                  out_specs=(PartitionSpec("core"),) * n_outs,
                  check_rep=False),
        donate_argnums=tuple(range(n_params, n_params + n_outs)),
        keep_unused=True,
    )

    def run(x_full: np.ndarray) -> np.ndarray:
        # x_full: [B, 64] int32 -> concat along rows is already the global
        # array; each core's shard is its contiguous row block.
        zeros = [np.zeros((NCORES * z.shape[0], *z.shape[1:]), z.dtype)
                 for z in zero_outs]
        out = sharded(x_full, *zeros)
        return np.asarray(out[0])

    run.sharded = sharded
    run.zero_outs = zero_outs
    run.mesh = mesh
    return run


def kernel(x: np.ndarray) -> np.ndarray:
    global _RUNNER
    x = np.asarray(x)
    assert x.shape == (B, L), x.shape
    if x.dtype != np.int32:
        x = x.astype(np.int32)
    if _RUNNER is None:
        _RUNNER = _build_runner()
    try:
        out = _RUNNER(x)
    except Exception:
        # transient device hiccups (NRT exec-unit resets) have been observed
        # once on this fabric; one retry after a short pause recovers.
        import time
        time.sleep(20.0)
        out = _RUNNER(x)
    return out.reshape(B, 1).astype(np.float32)


if __name__ == "__main__":
    rng = np.random.default_rng(0)
    xa = rng.integers(0, VOCAB, size=(B, L)).astype(np.int32)
    out = kernel(x=xa)
    # quick numpy check
    cnt = np.zeros((B, VOCAB), np.float64)
    for v in range(VOCAB):
        cnt[:, v] = (xa == v).sum(1)
    p = cnt / S_PRIME
    ref = -(p * np.log(p + EPS)).sum(1, keepdims=True)
    err = np.abs(out - ref).max()
    rel = err / np.abs(ref).max()
    print("selfcheck max abs err:", err, "rel:", rel)
